# revision 9
# baseline (speedup 1.0000x reference)
"""HGCN encoder forward on 8 Trainium2 NeuronCores.

Computation (per batch b):
    w_abs = |gelu(states @ W1.T + b1) @ W2.T + b2|          (E,)  [host, tiny]
    d[n]    = sum_e H[n,e] * w_abs[e]                        (N,)  [host]
    s[n,dd] = rsqrt(d[n]) * nf[n,dd]                         [host, 2 MiB]
    Y[dd,e] = sum_n s[n,dd] * H[n,e]                         [device]
    X[e,dd] = leaky_relu(w_abs[e] * (Y_half0 + Y_half1)[dd,e])  [host, tiny]

Sharding: core c -> (batch b = c//2, node-half c%2); each core owns
4096 node rows.  The device kernel is a pure DMA->PE stream, paced
entirely by the H read:

  * H is centered (H - 0.5 in [-0.5, 0.5)) and quantized to fp8 E3M4
    on the host.  In [-0.5, 0.5) e3m4's subnormal+first-normal bands
    give a uniform ~6-bit quantizer (max err 2^-7); measured output
    rel err ~7e-3 vs the 2e-2 gate.  The removed mean re-enters as a
    host-side rank-1 correction: Y += 0.5 * colsum(s).
  * 8.4 MiB/core streams via 16 deep-queued chunk DMAs (zero deps, all
    chunks SBUF-resident) at the ~400 GB/s HBM/fabric pace.
  * PE: per node-tile, the 4 e-chunk matmuls (K=128, M=16, N=512,
    lhsT = bf16 s-tile, rhs = fp8 H) are column-tiled to 32-col strips
    (tile_position=(0,32j)) so they run concurrently in the array;
    the PE tracks the DMA with ~3x headroom.  Accumulators sit in 4
    separate PSUM banks (a start=True matmul clears its whole bank, so
    groups may never share one).
  * DVE/ACT only drain PSUM at the end (partition-shifted copies,
    alternating engines across banks).

Host sums the two per-batch partials, adds the mean correction, scales
by w_abs, applies leaky_relu.
"""

import sys

for _p in ("/opt/trn_rl_repo",):
    if _p not in sys.path:
        sys.path.insert(0, _p)

import numpy as np

B, N, E, S, D = 4, 8192, 2048, 64, 16
NCORES = 8
NSHARD = N // 2          # nodes per core
NT = NSHARD // 128       # 32 node-tiles per core
ECH = 512                # e-chunk per matmul (one PSUM bank)
NJ = E // ECH            # 4 matmuls (banks) per node-tile
# The e-axis is split in two 1024-wide halves streamed back-to-back:
# half 0 feeds PSUM banks 0-1, half 1 feeds banks 2-3, so half-0's
# PSUM drain + output DMA hide completely under half-1's stream and
# only half-1's drain is exposed at the end.  Within a half, chunk
# sizes (in node-tiles; 1 tile = 1 KiB per partition line) start big
# for bandwidth and end small so the final receipt exposure is short.
EHALF = E // 2           # 1024 e-columns per half
CHUNK_TILES = [8, 8, 8, 4, 2, 1, 1]
assert sum(CHUNK_TILES) == NT
COLTILE = True           # concurrent 32-col-strip matmuls

_CACHE = {}


def _build_nc():
    import concourse.bass as bass  # noqa: F401
    import concourse.mybir as mybir
    import concourse.tile as tile
    from concourse import bacc

    f32 = mybir.dt.float32
    bf16 = mybir.dt.bfloat16
    f8 = mybir.dt.float8e3
    nc = bacc.Bacc(
        "TRN2",
        target_bir_lowering=False,
        debug=False,
        num_devices=NCORES,
    )
    hg = nc.dram_tensor("hg", [128, NT * E], f8, kind="ExternalInput").ap()
    sv = nc.dram_tensor("sv", [128, NT * D], bf16, kind="ExternalInput").ap()
    y = nc.dram_tensor("y", [D, E], f32, kind="ExternalOutput").ap()

    with tile.TileContext(nc) as tc:
        with (
            tc.tile_pool(name="hpool", bufs=1) as hpool,
            tc.tile_pool(name="wpool", bufs=1) as wpool,
            tc.tile_pool(name="psum", bufs=1, space="PSUM") as psum_pool,
        ):
            s_all = wpool.tile([128, NT * D], bf16, tag="sall")
            y_tile = wpool.tile([D, E], f32, tag="y")
            nc.sync.dma_start(s_all[:], sv[:])

            accs = [
                psum_pool.tile([128, ECH], f32, tag=f"acc{j}", name=f"acc{j}")
                for j in range(NJ)
            ]

            # DRAM col order: [half h][tile i][e' in 0..EHALF)
            for h in range(2):
                chunks = []
                base = 0
                for c, ctiles in enumerate(CHUNK_TILES):
                    h_c = hpool.tile([128, ctiles * EHALF], f8, tag=f"hg{h}_{c}")
                    off = (h * NT + base) * EHALF
                    nc.sync.dma_start(
                        h_c[:], hg[:, off : off + ctiles * EHALF]
                    )
                    chunks.append((h_c, base, ctiles))
                    base += ctiles

                for h_c, base, ctiles in chunks:
                    for t in range(ctiles):
                        i = base + t
                        for jj in range(2):
                            j = 2 * h + jj      # global bank / e-chunk index
                            if COLTILE:
                                out_ap = accs[j][32 * j : 32 * j + D, :]
                                tp = (0, 32 * j)
                            else:
                                out_ap = accs[j][0:D, :]
                                tp = None
                            rch = slice(
                                t * EHALF + jj * ECH,
                                t * EHALF + (jj + 1) * ECH,
                            )
                            nc.tensor.matmul(
                                out_ap,
                                lhsT=s_all[:, i * D : (i + 1) * D],
                                rhs=h_c[:, rch],
                                start=(i == 0),
                                stop=(i == NT - 1),
                                tile_position=tp,
                            )
                            if i == NT - 1:
                                ch = slice(j * ECH, (j + 1) * ECH)
                                src = (
                                    accs[j][32 * j : 32 * j + D, :]
                                    if COLTILE
                                    else accs[j][0:D, :]
                                )
                                if jj % 2 == 0:
                                    nc.scalar.copy(y_tile[:, ch], src)
                                else:
                                    nc.vector.tensor_copy(y_tile[:, ch], src)
                                if jj == 1:
                                    och = slice(h * EHALF, (h + 1) * EHALF)
                                    nc.sync.dma_start(
                                        y[:, och], y_tile[:, och]
                                    )

    nc.compile()
    return nc


def _get_nc():
    if "nc" not in _CACHE:
        _CACHE["nc"] = _build_nc()
    return _CACHE["nc"]


def _host_wabs(states, W1, b1, W2, b2):
    from scipy.special import erf

    st = states.astype(np.float64)
    h = st @ W1.astype(np.float64).T + b1.astype(np.float64)
    h = h * 0.5 * (1.0 + erf(h / np.sqrt(2.0)))
    w = h @ W2.astype(np.float64).T + b2.astype(np.float64)
    return np.abs(w).astype(np.float32)  # (B, E)


def _f32_to_bf16_u16(x):
    """Round-to-nearest-even f32 -> bf16, returned as a uint16 array."""
    u = np.ascontiguousarray(x, dtype=np.float32).view(np.uint32)
    r = ((u >> 16) & 1) + np.uint32(0x7FFF)
    return ((u + r) >> 16).astype(np.uint16)


def _make_in_maps(node_features, hyper_graph, w_abs):
    import ml_dtypes

    # degree + rsqrt + row-scale of node features, all on host (exact)
    s = np.empty((B, N, D), dtype=np.float32)
    for b in range(B):
        d = hyper_graph[b] @ w_abs[b]                      # (N,)
        dinv = np.where(
            d > 0, 1.0 / np.sqrt(d.astype(np.float64)), 0.0
        ).astype(np.float32)
        s[b] = dinv[:, None] * node_features[b]

    s_u16 = _f32_to_bf16_u16(s)
    # mean-correction uses the bf16-rounded s the device actually sees
    s_bf = s_u16.view(ml_dtypes.bfloat16).astype(np.float32)  # (B,N,D)
    s_sum = s_bf.sum(axis=1)                                  # (B,D)

    hq = (hyper_graph - np.float32(0.5)).astype(ml_dtypes.float8_e3m4)

    in_maps = []
    for c in range(NCORES):
        b, half = c // 2, c % 2
        sl = slice(half * NSHARD, (half + 1) * NSHARD)
        # cols: [half h][tile i][e' in 0..EHALF)
        hg_c = np.ascontiguousarray(
            hq[b, sl]
            .view(np.uint8)
            .reshape(NT, 128, 2, EHALF)
            .transpose(1, 2, 0, 3)
        ).reshape(128, NT * E).view(ml_dtypes.float8_e3m4)
        s_c = np.ascontiguousarray(
            s_u16[b, sl].reshape(NT, 128, D).transpose(1, 0, 2)
        ).reshape(128, NT * D).view(ml_dtypes.bfloat16)
        in_maps.append({"hg": hg_c, "sv": s_c})
    return in_maps, s_sum


def kernel(**inputs):
    from concourse.bass_utils import run_bass_kernel_spmd

    node_features = np.asarray(inputs["node_features"], dtype=np.float32)
    hyper_graph = np.asarray(inputs["hyper_graph"], dtype=np.float32)
    states = np.asarray(inputs["states"], dtype=np.float32)
    W1 = np.asarray(inputs["W1"], dtype=np.float32)
    b1 = np.asarray(inputs["b1"], dtype=np.float32)
    W2 = np.asarray(inputs["W2"], dtype=np.float32)
    b2 = np.asarray(inputs["b2"], dtype=np.float32)

    w_abs = _host_wabs(states, W1, b1, W2, b2)
    in_maps, s_sum = _make_in_maps(node_features, hyper_graph, w_abs)

    nc = _get_nc()
    res = run_bass_kernel_spmd(nc, in_maps, core_ids=list(range(NCORES)))

    X = np.empty((B, E, D), dtype=np.float32)
    for b in range(B):
        p = res.results[2 * b]["y"] + res.results[2 * b + 1]["y"]  # (D, E)
        p = p + np.float32(0.5) * s_sum[b][:, None]                # mean corr
        xb = (p * w_abs[b][None, :]).T                             # (E, D)
        X[b] = np.where(xb >= 0, xb, np.float32(0.1) * xb)
    return X
